# revision 75
# baseline (speedup 1.0000x reference)
"""Multi-head cross-attention on 8 Trainium2 NeuronCores.

Problem shapes (hardcoded): B=4, Ld=1024, Le=2048, d_model=1024, 8 heads x 128.
Sharding: core c handles batch b=c//2 and head-group g=c%2 (4 heads each).
Each core computes q/k/v projections for its heads, attention, and a partial
output projection over its heads' value dims; the host sums the two partial
outputs per batch and adds the bias.

The attention path runs in bf16 (full PE rate); the Q/K/V projections run as
scaled hi+lo fp8 DoubleRow matmuls: each operand X is sent as fp8(s*X) plus
fp8(s*X - fp8(s*X)) packed [part, d-chunk, hi|lo, col], and each projection
accumulates the three cross terms (hi*hi + hi*lo + lo*hi) over 256-wide
DoubleRow contractions — bf16-class accuracy at 3/4 of the bf16 PE cost.
The 1/(8*64) operand descale is folded into the PSUM-drain copies, and the
softmax 1/sqrt(dh) into the exp activation's scale immediate.
Exact algebraic reductions vs the reference:
  - the k bias is dropped: adding q.bk to every score of a query cancels in
    softmax,
  - the v bias folds into a constant output bias (attention weights sum to 1,
    so attn@(v+bv) = attn@v + bv), applied host-side together with b_o,
  - the softmax denominator is computed by the same matmuls as attn@V: the
    moving operand is [v_chunk | ones-column] (129 wide) with exp'd scores as
    the stationary operand, so column 128 of the accumulator is sum(exp) and
    no separate denominator pass is needed.
The fused attn@V produces vals in [q, vd] layout; a DMA-XBAR transpose turns
it into [vd, q] for the output projection, keeping the PE free.

Work is emitted software-pipelined in units of (head, query-half).  Engine
queues are strict FIFO, so emission order is chosen so that no instruction
ever waits on one emitted later: V-projection chunks are front-loaded as
filler behind units 0-1's scores, unit u's fused matmuls are spread as filler
one to two units later (inside the pT pool's 24-tile reuse deadline, before
a later unit's exp can throttle on its WAR edge), and the qh0 output
projection fills units 5-7 with the drain ordered so qh1 transposes have
landed before their outproj consumers.
"""

import math
import sys
from collections import deque

import numpy as np

for _p in ("/opt/trn_rl_repo", "/root/.axon_site/_ro/trn_rl_repo"):
    if _p not in sys.path:
        sys.path.append(_p)

B = 4
LQ = 1024
LK = 2048
D = 1024
H = 8
DH = 128
P = 128
HPC = 4          # heads per core
OQ = HPC * DH    # 512 projected dims per core
KC = D // P      # 8 contraction chunks for projections
LKC = LK // P    # 16 key chunks
QH = 512         # query half
N_CORES = 8

_BUILT = {}


def _build(masked):
    import concourse.bass as bass  # noqa: F401
    import concourse.tile as tile
    import concourse.mybir as mybir
    from concourse import bacc

    f32 = mybir.dt.float32
    bf16 = mybir.dt.bfloat16
    fp8 = mybir.dt.float8e4
    Exp = mybir.ActivationFunctionType.Exp
    Ident = mybir.ActivationFunctionType.Identity
    DR = mybir.MatmulPerfMode.DoubleRow
    Mult = mybir.AluOpType.mult
    AddOp = mybir.AluOpType.add
    DS = 1.0 / 512.0         # descale for the x8/x64 hi-lo fp8 operands
    TERMS = ((0, 0), (0, 1), (1, 0))   # (weights, activations) hi/lo terms

    nc = bacc.Bacc("TRN2", target_bir_lowering=False, debug=False,
                   num_devices=N_CORES)

    enc8T = nc.dram_tensor("enc8T", [P, KC, 2, LK], fp8,
                           kind="ExternalInput").ap()
    x8T = nc.dram_tensor("x8T", [P, KC, 2, LQ], fp8,
                         kind="ExternalInput").ap()
    wk8T = nc.dram_tensor("wk8T", [P, KC, 2, OQ], fp8,
                          kind="ExternalInput").ap()
    wv8T = nc.dram_tensor("wv8T", [P, KC, 2, OQ], fp8,
                          kind="ExternalInput").ap()
    wq8T = nc.dram_tensor("wq8T", [P, KC, 2, OQ], fp8,
                          kind="ExternalInput").ap()
    woT = nc.dram_tensor("woT", [OQ, D], bf16, kind="ExternalInput").ap()
    bq_d = nc.dram_tensor("bq", [P, HPC], f32, kind="ExternalInput").ap()
    if masked:
        maskT = nc.dram_tensor("maskT", [LK, LQ], bf16,
                               kind="ExternalInput").ap()
    out_d = nc.dram_tensor("out", [LQ, D], bf16, kind="ExternalOutput").ap()

    with tile.TileContext(nc) as tc:
        with tc.tile_pool(name="persist", bufs=1) as persist:
            # hi/lo fp8 operand pairs, packed as [part, d-chunk, hi|lo, col]
            e8 = [persist.tile([P, KC, 2, 1024], fp8, name=f"e8_{lh}")
                  for lh in range(2)]
            x8 = persist.tile([P, KC, 2, LQ], fp8, name="x8")
            wk8 = persist.tile([P, KC, 2, OQ], fp8, name="wk8")
            wv8 = persist.tile([P, KC, 2, OQ], fp8, name="wv8")
            wq8 = persist.tile([P, KC, 2, OQ], fp8, name="wq8")
            woch = [persist.tile([P, D], bf16, name=f"wo{h}")
                    for h in range(HPC)]
            kT = [persist.tile([P, LK], bf16, name=f"kT{h}") for h in range(HPC)]
            qT = [persist.tile([P, LQ], bf16, name=f"qT{h}") for h in range(HPC)]
            # per key-chunk: 4 head blocks of [128 v-dims | ones | pad]
            vch = [persist.tile([P, HPC, 130], bf16, name=f"v{j}")
                   for j in range(LKC)]
            valsT = [persist.tile([P, LQ], bf16, name=f"valsT{h}")
                     for h in range(HPC)]
            bq_sb = persist.tile([P, HPC], f32, name="bq")
            warm = persist.tile([P, 1], f32, name="warm")

            # touch the ACT engine immediately so its activation-table load
            # happens in the cold-start DMA shadow, not in front of the
            # first real ACT drain (which gates PSUM bank reuse)
            nc.vector.memset(warm[:], 0.0)
            nc.scalar.mul(warm[:], warm[:], 1.0)

            # ---- input DMAs, in consumption order.  K's weights go on
            # the SP HWDGE path and enc chunks on the gpsimd SWDGE path so
            # descriptor generation pipelines in parallel and the K half-0
            # stream is transfer-paced only.
            # the first DoubleRow matmul needs chunks 0 AND 1: spread their
            # DMAs across both descriptor paths so they land together
            nc.sync.dma_start(e8[0][:, 0, :, :], enc8T[:, 0, :, :1024])
            nc.gpsimd.dma_start(e8[0][:, 1, :, :], enc8T[:, 1, :, :1024])
            nc.sync.dma_start(wk8[:, 0:2, :, :], wk8T[:, 0:2, :, :])
            nc.sync.dma_start(bq_sb[:], bq_d[:])
            for d in range(2, KC):
                if d % 2 == 0:
                    nc.sync.dma_start(wk8[:, d:d + 2, :, :],
                                      wk8T[:, d:d + 2, :, :])
                nc.gpsimd.dma_start(e8[0][:, d, :, :],
                                    enc8T[:, d, :, :1024])
            for d in range(KC):
                # all of half 1 on the fast HWDGE path: its transfers must
                # queue ahead of the big wq8/x8/wv8 loads on the shared
                # transfer device
                nc.sync.dma_start(e8[1][:, d, :, :], enc8T[:, d, :, 1024:])
            nc.sync.dma_start(wq8[:], wq8T[:, :, :, :])
            nc.gpsimd.dma_start(x8[:], x8T[:, :, :, :])
            nc.sync.dma_start(wv8[:], wv8T[:, :, :, :])
            for h in range(HPC):
                nc.sync.dma_start(woch[h][:], woT[h * P:(h + 1) * P, :])

            # ones columns for the fused attn@V / denominator matmuls
            for j in range(LKC):
                nc.vector.memset(vch[j][:], 1.0)

            # ---- K projection then Q projection.  Phase 1 owns all 8 PSUM
            # banks, so each K half runs as ONE 8-group pass (2 quarters x 4
            # heads): every arriving enc chunk feeds 8 matmuls, keeping the
            # PE ahead of the 360 GB/s transfer stream from the first chunk.
            with tc.tile_pool(name="ppA", bufs=1, space="PSUM") as ppA:
                ab = [ppA.tile([P, 512], f32, name=f"a{t}") for t in range(8)]
                for lh in range(2):
                    for dp in range(4):          # d-chunk pairs (DoubleRow)
                        for t in range(8):
                            lkq, hh = t // 4, t % 4
                            for ti, (wi, ai) in enumerate(TERMS):
                                nc.tensor.matmul(
                                    ab[t][:],
                                    wk8[:, 2 * dp:2 * dp + 2, wi,
                                        hh * DH:(hh + 1) * DH],
                                    e8[lh][:, 2 * dp:2 * dp + 2, ai,
                                           lkq * 512:(lkq + 1) * 512],
                                    start=(dp == 0 and ti == 0),
                                    stop=(dp == 3 and ti == 2),
                                    perf_mode=DR)
                            if dp == 3:
                                # drain (with the 1/512 hi-lo descale) right
                                # behind the closing matmul, alternating DVE
                                # and ACT so the eight drains retire in half
                                # the time (the next phase's matmuls wait on
                                # these banks)
                                quarter = lh * 2 + lkq
                                dst = kT[hh][:, quarter * 512:
                                             (quarter + 1) * 512]
                                if t % 2 == 0:
                                    nc.vector.tensor_scalar_mul(
                                        dst, ab[t][:], DS)
                                else:
                                    nc.scalar.mul(dst, ab[t][:], DS)
                for qh in range(2):
                    for dp in range(4):
                        for h in range(HPC):
                            for ti, (wi, ai) in enumerate(TERMS):
                                nc.tensor.matmul(
                                    ab[(1 - qh) * 4 + h][:],
                                    wq8[:, 2 * dp:2 * dp + 2, wi,
                                        h * DH:(h + 1) * DH],
                                    x8[:, 2 * dp:2 * dp + 2, ai,
                                       qh * QH:(qh + 1) * QH],
                                    start=(dp == 0 and ti == 0),
                                    stop=(dp == 3 and ti == 2),
                                    perf_mode=DR)
                            if dp == 3:
                                # descale + bias add, split across DVE and
                                # the idle ACT engine so the pool drain that
                                # gates the first scores matmul is short
                                dst = qT[h][:, qh * QH:(qh + 1) * QH]
                                src = ab[(1 - qh) * 4 + h][:]
                                if h % 2 == 0:
                                    nc.vector.tensor_scalar(
                                        dst, src, DS, bq_sb[:, h:h + 1],
                                        Mult, AddOp)
                                else:
                                    nc.scalar.activation(
                                        dst, src, Ident,
                                        bias=bq_sb[:, h:h + 1], scale=DS)

            # ---- attention + V projection + output projection, pipelined.
            with (
                tc.tile_pool(name="pTp", bufs=24) as pTp,
                tc.tile_pool(name="vsb", bufs=8) as vsbp,
                tc.tile_pool(name="rsb", bufs=8) as rsbp,
                tc.tile_pool(name="osb", bufs=4) as osbp,
                tc.tile_pool(name="msk", bufs=8 if masked else 1) as mskp,
                tc.tile_pool(name="ppS", bufs=1, space="PSUM") as ppS,
                tc.tile_pool(name="ppF", bufs=1, space="PSUM") as ppF,
            ):
                st = [ppS.tile([P, 1024], f32, name=f"s{t}") for t in range(2)]
                ft = [ppF.tile([P, 129], f32, name=f"f{t}") for t in range(2)]

                unit_pts = {}   # unit -> list of 8 pT tiles
                sg_counter = [0]

                def emit_scores_group(u, g):
                    """Two scores matmuls (chunks 2g, 2g+1) + one exp."""
                    h, qh = u % HPC, u // HPC
                    s = st[sg_counter[0] % 2]
                    sg_counter[0] += 1
                    for jj in range(2):
                        j = g * 2 + jj
                        nc.tensor.matmul(
                            s[:, jj * 512:(jj + 1) * 512],
                            kT[h][:, j * P:(j + 1) * P],
                            qT[h][:, qh * QH:(qh + 1) * QH],
                            start=True, stop=True)
                        if masked:
                            mt = mskp.tile([P, 512], bf16, name="m")
                            nc.sync.dma_start(
                                mt[:], maskT[j * P:(j + 1) * P,
                                             qh * QH:(qh + 1) * QH])
                            nc.vector.tensor_add(
                                s[:, jj * 512:(jj + 1) * 512],
                                s[:, jj * 512:(jj + 1) * 512], mt[:])
                    pt = pTp.tile([P, 1024], bf16, name="pt")
                    nc.scalar.activation(pt[:], s[:], Exp,
                                         scale=0.08838834764831845)
                    unit_pts.setdefault(u, []).append(pt)

                def emit_fused_quarter(u, qs, quarter):
                    """4 fused attn@V+denominator matmuls (one j-quarter of
                    the 16-chunk accumulation); normalize + transpose after
                    the last one."""
                    h, qh = u % HPC, u // HPC
                    f = ft[qs % 2]
                    pts = unit_pts[u]
                    for j in range(quarter * 4, quarter * 4 + 4):
                        g, jj = j // 2, j % 2
                        nc.tensor.matmul(
                            f[:],
                            pts[g][:, jj * 512 + qs * P:jj * 512 + (qs + 1) * P],
                            vch[j][:, h, 0:129],
                            start=(j == 0), stop=(j == LKC - 1))
                    if quarter == 3:
                        rc = rsbp.tile([P, 1], f32, name="rc")
                        nc.vector.reciprocal(rc[:], f[:, 128:129])
                        vs = vsbp.tile([P, P], bf16, name="vs")
                        nc.vector.tensor_scalar_mul(vs[:], f[:, 0:128], rc[:])
                        nc.sync.dma_start(
                            valsT[h][:, qh * QH + qs * P:qh * QH + (qs + 1) * P],
                            vs[:], transpose=True)

                def emit_vproj_half(j, half, vtile):
                    """Half of the hi-lo DoubleRow V projection for key
                    chunk j (2 of 4 d-pairs); descale+copy at the end."""
                    lh, jloc = j // 8, j % 8
                    for dp in range(half * 2, half * 2 + 2):
                        for ti, (wi, ai) in enumerate(TERMS):
                            nc.tensor.matmul(
                                vtile[:],
                                e8[lh][:, 2 * dp:2 * dp + 2, ai,
                                       jloc * P:(jloc + 1) * P],
                                wv8[:, 2 * dp:2 * dp + 2, wi, :],
                                start=(dp == 0 and ti == 0),
                                stop=(dp == 3 and ti == 2),
                                perf_mode=DR)
                    if half == 1:
                        nc.vector.tensor_scalar_mul(
                            vch[j][:, :, 0:128], vtile[:], DS)

                def emit_outproj_group(qh, lqc, oh, otile, obuf):
                    """Output projection for one (q-chunk, out-half)."""
                    for h in range(HPC):
                        nc.tensor.matmul(
                            otile[:],
                            valsT[h][:, qh * QH + lqc * P:
                                      qh * QH + (lqc + 1) * P],
                            woch[h][:, oh * 512:(oh + 1) * 512],
                            start=(h == 0), stop=(h == HPC - 1))
                    nc.vector.tensor_copy(obuf[:, oh * 512:(oh + 1) * 512],
                                          otile[:])
                    lq = qh * 4 + lqc
                    if qh == 1:
                        # per-half DMAs at the tail so the last transfer is
                        # small and starts as soon as its copy lands
                        nc.sync.dma_start(
                            out_d[lq * P:(lq + 1) * P,
                                  oh * 512:(oh + 1) * 512],
                            obuf[:, oh * 512:(oh + 1) * 512])
                    elif oh == 1:
                        nc.sync.dma_start(
                            out_d[lq * P:(lq + 1) * P, :], obuf[:])

                og_counter = [0]
                obufs = {}

                def out_args(qh, lqc, oh):
                    key = (qh, lqc)
                    if oh == 0:
                        obufs[key] = osbp.tile([P, D], bf16, name="ob")
                    otile = out_tiles[og_counter[0] % 2]
                    og_counter[0] += 1
                    return otile, obufs[key]

                def emit_item(item):
                    if item[0] == "v":
                        emit_vproj_half(item[1], item[2], item[3])
                        return 640
                    if item[0] == "f":
                        emit_fused_quarter(item[1], item[2], item[3])
                        return 215
                    _, qh, lqc, oh = item
                    emit_outproj_group(qh, lqc, oh,
                                       *out_args(qh, lqc, oh))
                    return 853

                def run_unit(u, items):
                    """Emit unit u's 8 scores groups with `items` spread
                    across the 8 slots proportionally by estimated time."""
                    total = sum({"v": 640, "f": 215, "o": 853}[i[0]]
                                for i in items)
                    items = deque(items)
                    done = 0
                    for g in range(8):
                        emit_scores_group(u, g)
                        target = total * (g + 1) // 8
                        while items and done < target:
                            done += emit_item(items.popleft())

                def fq(u, qss, quarters):
                    return [("f", u, qs, q) for qs in qss for q in quarters]

                # ---- phase 2: units 0-3 (qh0); V projection as filler
                # (front-loaded: fused quarters need vch complete), then
                # F_0/F_1 spread behind units 2-3.
                with tc.tile_pool(name="ppV", bufs=1, space="PSUM") as ppV:
                    vt = [ppV.tile([P, HPC, 128], f32, name=f"v{t}")
                          for t in range(2)]
                    vitems = [("v", j, half, vt[j % 2])
                              for j in range(LKC) for half in range(2)]
                    run_unit(0, vitems[:14])
                    run_unit(1, vitems[14:28])
                    run_unit(2, vitems[28:] + fq(0, range(4), range(4)))
                    run_unit(3, fq(1, range(4), range(4)) +
                             fq(2, (0, 1), range(4)))

                # ---- phase 3: units 4-7 (qh1) + output projection.
                with tc.tile_pool(name="ppO", bufs=1, space="PSUM") as ppO:
                    out_tiles = [ppO.tile([P, 512], f32, name=f"o{t}")
                                 for t in range(2)]
                    run_unit(4, fq(2, (2, 3), range(4)) +
                             fq(3, range(4), range(4)))
                    run_unit(5, [("o", 0, 0, 0), ("o", 0, 0, 1)] +
                             fq(4, range(4), range(4)))
                    run_unit(6, [("o", 0, 1, 0), ("o", 0, 1, 1)] +
                             fq(5, range(4), range(4)))
                    run_unit(7, [("o", 0, 2, 0), ("o", 0, 2, 1)] +
                             fq(6, range(4), range(4)))
                    # drain: F_7, a reserved qh0 group to cover qh1 transpose
                    # latency, then qh1 outproj.
                    for item in fq(7, range(4), range(4)):
                        emit_item(item)
                    for oh in range(2):
                        emit_item(("o", 0, 3, oh))
                    for lqc in range(4):
                        for oh in range(2):
                            emit_item(("o", 1, lqc, oh))

    nc.compile()
    return nc


def _get_built(masked):
    if masked not in _BUILT:
        _BUILT[masked] = _build(masked)
    return _BUILT[masked]


def _shard_inputs(inputs, masked):
    import ml_dtypes
    bf16 = ml_dtypes.bfloat16
    fp8 = ml_dtypes.float8_e4m3

    x = np.asarray(inputs["mhca_input"], np.float32)
    enc = np.asarray(inputs["encoder_output"], np.float32)
    mask = np.asarray(inputs["cross_mask"], np.float32)
    W_kv = np.asarray(inputs["W_kv"], np.float32)
    W_q = np.asarray(inputs["W_q"], np.float32)
    b_q = np.asarray(inputs["b_q"], np.float32)
    W_o = np.asarray(inputs["W_o"], np.float32)

    def pack_hilo(a, s):
        """[D, C] fp32 -> [P, KC, 2, C] fp8 hi/lo pair of a*s."""
        b = a * s
        hi = b.astype(fp8)
        lo = (b - hi.astype(np.float32)).astype(fp8)
        arr = np.stack([hi, lo], 1)               # [D, 2, C]
        arr = arr.reshape(KC, P, 2, a.shape[1])
        return np.ascontiguousarray(arr.transpose(1, 0, 2, 3))

    in_maps = []
    for c in range(N_CORES):
        b = c // 2
        g = c % 2
        heads = list(range(g * HPC, (g + 1) * HPC))
        sl = slice(g * OQ, (g + 1) * OQ)
        k_rows = np.concatenate(
            [W_kv[h * 2 * DH:h * 2 * DH + DH] for h in heads], 0)
        v_rows = np.concatenate(
            [W_kv[h * 2 * DH + DH:(h + 1) * 2 * DH] for h in heads], 0)
        m = {
            "enc8T": pack_hilo(enc[b].T, 8.0),
            "x8T": pack_hilo(x[b].T, 8.0),
            "wk8T": pack_hilo(k_rows.T, 64.0),
            "wv8T": pack_hilo(v_rows.T, 64.0),
            "wq8T": pack_hilo(W_q[sl].T, 64.0),
            "woT": np.ascontiguousarray(W_o[:, sl].T).astype(bf16),
            # raw bias: the 1/sqrt(dh) now lives in the exp scale
            "bq": np.ascontiguousarray(b_q[sl].reshape(HPC, DH).T),
        }
        if masked:
            # scores reach exp un-scaled; pre-multiply the mask to match
            m["maskT"] = np.ascontiguousarray(
                mask[b].T * math.sqrt(DH)).astype(bf16)
        in_maps.append(m)
    return in_maps


def kernel(mhca_input, encoder_output, cross_mask, W_kv, b_kv, W_q, b_q, W_o,
           b_o):
    from concourse.bass_utils import run_bass_kernel_spmd

    inputs = {
        "mhca_input": mhca_input, "encoder_output": encoder_output,
        "cross_mask": cross_mask, "W_kv": W_kv, "b_kv": b_kv, "W_q": W_q,
        "b_q": b_q, "W_o": W_o,
    }
    b_kv = np.asarray(b_kv, np.float32)
    b_o = np.asarray(b_o, np.float32)
    W_o_np = np.asarray(W_o, np.float32)
    masked = bool(np.any(np.asarray(cross_mask)))
    nc = _get_built(masked)
    in_maps = _shard_inputs(inputs, masked)

    res = run_bass_kernel_spmd(nc, in_maps, core_ids=list(range(N_CORES)))
    outs = [np.asarray(res.results[c]["out"], np.float32)
            for c in range(N_CORES)]
    full = np.stack([outs[2 * b] + outs[2 * b + 1] for b in range(B)], 0)
    # v-bias folds into a constant output bias: attn@(v+bv) = attn@v + bv.
    b_v = np.concatenate([b_kv[h * 2 * DH + DH:(h + 1) * 2 * DH]
                          for h in range(H)], 0)
    bias = b_o + W_o_np @ b_v
    return (full + bias[None, None, :]).astype(np.float32)
